# revision 29
# baseline (speedup 1.0000x reference)
"""Trainium2 Bass kernel for nn_Attention_24919400252009.

Multi-head attention (8 heads, head_dim 32) over B=2,G=4,Q=K=1024,C=256,
returning (final_output, attn_probs).  Data-parallel across the 8
NeuronCores: one (b, g) pair per core, no collectives.

Per-core pipeline (bf16 compute, f32 accumulation), software-pipelined
across heads (stage T/D/A of head h-1 is emitted with stage S/E/M/N of
head h so every engine's in-order stream stays dense):
  S: PE scores_qk = qT_h.T @ kT_h    (contraction d=32, row-group packed)
  E: ACT e = exp(scores / sqrt(32))  (PSUM -> SBUF bf16)
  M: DVE scalar_tensor_tensor: em = e * mask, row-sums accum (f32)
  N: DVE tensor_scalar: probs = em * (1/sum)  (bf16, in place)
  T: HWDGE xbar transpose probs [q,k] -> [k,q] bf16
  D: SWDGE cast-DMA probs bf16 -> HBM f32 (attn_probs output)
  A: PE out_h = v_h.T @ probsT (col-group packed) -> Wo projection
"""

import sys

if "/opt/trn_rl_repo" not in sys.path:
    sys.path.insert(0, "/opt/trn_rl_repo")

import numpy as np
import ml_dtypes

B, G, QL, KL = 2, 4, 1024, 1024
C = 256          # CQ = CKV = QK_CH = V_CH = OUT_CH
H = 8            # num heads
HD = 32          # head dim
P = 128          # partitions
NQT = QL // P    # 8 q tiles
NKT = KL // P    # 8 k tiles
NH = QL // 512   # matmul free-dim halves (PSUM bank limit N <= 512)
SCALE = 1.0 / np.sqrt(HD)

BF16 = ml_dtypes.bfloat16

_CACHED = {}


def _build_nc(repeats=1, serial=False, accum_bf16=False):
    import concourse.mybir as mybir
    import concourse.tile as tile
    from concourse import bacc
    from concourse.bass import ts

    nc = bacc.Bacc()
    bf = mybir.dt.bfloat16
    f32 = mybir.dt.float32

    # ---- DRAM I/O (per-core shard shapes) ----
    xqT_d = nc.dram_tensor("xqT", [C, QL], bf, kind="ExternalInput")
    xkvT_d = nc.dram_tensor("xkvT", [C, KL], bf, kind="ExternalInput")
    mask_d = nc.dram_tensor("maskq", [QL, KL], bf, kind="ExternalInput")
    w4_d = nc.dram_tensor("W4", [C, 4 * C], bf, kind="ExternalInput")  # q,k,v,o
    b3_d = nc.dram_tensor("B3", [1, 3 * C], bf, kind="ExternalInput")  # q,k,v
    outf_d = nc.dram_tensor("out_final", [QL, C], f32, kind="ExternalOutput")
    outp_d = nc.dram_tensor("out_probs", [H, QL, KL], f32, kind="ExternalOutput")

    with tile.TileContext(nc) as tc:
        with (
            tc.tile_pool(name="const", bufs=1) as cpool,
            tc.tile_pool(name="work", bufs=2) as wpool,
            tc.tile_pool(name="psum", bufs=2, space="PSUM") as ppool,
        ):
            # ---- load inputs (HWDGE; mask last - not needed until M(0)) ----
            xqT = cpool.tile([P, 2, QL], bf)       # [c_in, c_chunk, q]
            xkvT = cpool.tile([P, 2, KL], bf)
            w4 = cpool.tile([P, 2, 4 * C], bf)     # [c_in, c_chunk, 4*ch_out]
            b3 = cpool.tile([1, 3 * C], bf)
            mask = cpool.tile([P, NQT, KL], bf)    # [q_in_tile, q_tile, k]
            ones = cpool.tile([1, QL], bf)

            nc.sync.dma_start(xqT[:], xqT_d[:].rearrange("(cc p) q -> p cc q", p=P))
            nc.sync.dma_start(w4[:], w4_d[:].rearrange("(cc p) o -> p cc o", p=P))
            nc.sync.dma_start(xkvT[:], xkvT_d[:].rearrange("(cc p) q -> p cc q", p=P))
            nc.sync.dma_start(b3[:], b3_d[:])
            nc.sync.dma_start(mask[:], mask_d[:].rearrange("(qt p) k -> p qt k", p=P))
            nc.vector.memset(ones[:], 1.0)

            wqT = w4[:, :, 0 * C : 1 * C]
            wkT = w4[:, :, 1 * C : 2 * C]
            wvT = w4[:, :, 2 * C : 3 * C]
            woT = w4[:, :, 3 * C : 4 * C]

            # ---- projections ----
            # qT/kT: [ch_out(2x128), q] = W @ x.T (+ bias via rank-1 matmul)
            qT = cpool.tile([P, 2, QL], bf)
            kT = cpool.tile([P, 2, KL], bf)
            for i, (dst, w, x) in enumerate(((qT, wqT, xqT), (kT, wkT, xkvT))):
                for oc in range(2):  # output-channel chunk
                    ps_p = ppool.tile([P, QL], mybir.dt.float32, tag="ps_a")
                    for nh in range(NH):
                        for cc in range(2):  # contraction chunk
                            nc.tensor.matmul(
                                ps_p[:, ts(nh, 512)],
                                w[:, cc, ts(oc, P)],
                                x[:, cc, ts(nh, 512)],
                                start=(cc == 0),
                                stop=False,
                            )
                        nc.tensor.matmul(
                            ps_p[:, ts(nh, 512)],
                            b3[:, i * C + oc * P : i * C + (oc + 1) * P],
                            ones[:, ts(nh, 512)],
                            start=False, stop=True,
                        )
                    nc.scalar.copy(dst[:, oc, :], ps_p[:])

            # v: [k(8x128), ch_out] = x_kv @ Wv.T (+ bias via rank-1 matmul)
            v = cpool.tile([P, NKT, C], bf)
            for kt in range(NKT):
                ps_v = ppool.tile([P, QL], mybir.dt.float32, tag="ps_a")
                for cc in range(2):
                    nc.tensor.matmul(
                        ps_v[:, :C],
                        xkvT[:, cc, ts(kt, P)],
                        wvT[:, cc, :],
                        start=(cc == 0),
                        stop=False,
                    )
                nc.tensor.matmul(
                    ps_v[:, :C], ones[:, :P], b3[:, 2 * C : 2 * C + C],
                    start=False, stop=True,
                )
                nc.scalar.copy(v[:, kt, :], ps_v[:, :C])

            # ---- attention: software-pipelined head loop ----
            outT = cpool.tile([P, 2, QL], bf)  # [d(4x32 per chunk), chunk, q]
            state = {}

            def stage_semn(h):
                """scores -> exp -> mask+sums -> recip -> normalize.

                recip/normalize run per half-head so the downstream
                transpose + output DMA can start before the second half's
                mask pass finishes.
                """
                hc, hp = h // 4, (h % 4) * HD
                hq = NQT // 2
                e = wpool.tile([P, NQT, KL], bf, tag="e", name=f"e{h}", bufs=3)
                sum_dt = bf if accum_bf16 else mybir.dt.float32
                sums = wpool.tile([P, NQT], sum_dt, tag="sums",
                                  name=f"sums{h}")
                rinv = wpool.tile([P, NQT], mybir.dt.float32, tag="rinv",
                                  name=f"rinv{h}")
                for qt in range(NQT):
                    ps_s = ppool.tile([P, KL], mybir.dt.float32, tag="ps_a",
                                      name=f"ps_s{h}_{qt}")
                    for nh in range(NH):
                        nc.tensor.matmul(
                            ps_s[:, ts(nh, 512)],
                            qT[hp : hp + HD, hc, ts(qt, P)],
                            kT[hp : hp + HD, hc, ts(nh, 512)],
                            start=True, stop=True,
                            tile_position=(hp, 0),
                        )
                    nc.scalar.activation(
                        e[:, qt, :], ps_s[:],
                        mybir.ActivationFunctionType.Exp,
                        scale=float(SCALE),
                    )
                    nc.vector.scalar_tensor_tensor(
                        out=e[:, qt, :],
                        in0=e[:, qt, :],
                        scalar=1.0,
                        in1=mask[:, qt, :],
                        op0=mybir.AluOpType.mult,
                        op1=mybir.AluOpType.mult,
                        accum_out=sums[:, qt : qt + 1],
                    )
                    if qt % hq == hq - 1:
                        ch = qt // hq
                        sl = slice(ch * hq, (ch + 1) * hq)
                        nc.vector.reciprocal(rinv[:, sl], sums[:, sl])
                        for q2 in range(ch * hq, (ch + 1) * hq):
                            nc.vector.tensor_scalar_mul(
                                e[:, q2, :], e[:, q2, :],
                                rinv[:, q2 : q2 + 1],
                            )
                return e

            def stage_td(h, e):
                """xbar transposes + attn_probs cast-DMA out."""
                pT = wpool.tile([P, NQT, NKT, P], bf, tag="pT", name=f"pT{h}",
                                bufs=3)
                hq = NQT // 2
                for qt in range(NQT):
                    nc.sync.dma_start_transpose(pT[:, qt, :, :], e[:, qt, :])
                    if qt % hq == hq - 1:
                        # attn_probs out: bf16 -> f32 cast during DMA (SWDGE)
                        ch = qt // hq
                        nc.gpsimd.dma_start(
                            outp_d[h][ch * hq * P : (ch + 1) * hq * P].rearrange(
                                "(qt p) k -> p qt k", p=P
                            ),
                            e[:, ch * hq : (ch + 1) * hq, :],
                        )
                return pT

            def stage_a(h, pT):
                """AV matmuls (per q-block accumulation) + group drain."""
                hc, hp = h // 4, (h % 4) * HD
                if h % 4 == 0:
                    state["ps_o"] = ppool.tile(
                        [P, QL], mybir.dt.float32, tag="ps_o", name=f"ps_o{hc}"
                    )
                ps_o = state["ps_o"]
                for qt in range(NQT):
                    for kt in range(NKT):
                        nc.tensor.matmul(
                            ps_o[hp : hp + HD, ts(qt, P)],
                            v[:, kt, h * HD : (h + 1) * HD],
                            pT[:, qt, kt, :],
                            start=(kt == 0), stop=(kt == NKT - 1),
                            tile_position=(0, hp),
                        )
                if h % 4 == 3:
                    nc.scalar.copy(outT[:, hc, :], ps_o[:])

            for _rep in range(repeats):
                if serial and _rep:
                    tc.strict_bb_all_engine_barrier()
                for i in range(H + 2):
                    if i < H:
                        state[("e", i)] = stage_semn(i)
                    if 1 <= i <= H:
                        state[("pT", i - 1)] = stage_td(
                            i - 1, state.pop(("e", i - 1))
                        )
                    if i >= 2:
                        stage_a(i - 2, state.pop(("pT", i - 2)))

                # ---- final projection: outF[q, ch] = outT.T @ WoT ----
                outF = cpool.tile([P, NQT, C], f32, tag="outF")
                for qt in range(NQT):
                    ps_f = ppool.tile([P, QL], mybir.dt.float32, tag="ps_a")
                    for cc in range(2):
                        nc.tensor.matmul(
                            ps_f[:, :C],
                            outT[:, cc, ts(qt, P)],
                            woT[:, cc, :],
                            start=(cc == 0),
                            stop=(cc == 1),
                        )
                    nc.scalar.copy(outF[:, qt, :], ps_f[:, :C])
                nc.gpsimd.dma_start(
                    outf_d[:].rearrange("(qt p) c -> p qt c", p=P), outF[:]
                )

    nc.compile()
    return nc


def _get_nc():
    if "nc" not in _CACHED:
        _CACHED["nc"] = _build_nc()
    return _CACHED["nc"]


def _make_in_maps(inputs_q, inputs_kv, attention_mask, Wq, bq, Wk, bk, Wv, bv, Wo):
    w4 = np.concatenate([Wq.T, Wk.T, Wv.T, Wo.T], axis=1)
    w4 = np.ascontiguousarray(w4).astype(BF16)  # [c_in, 4*ch_out]
    b3 = np.concatenate([bq, bk, bv]).reshape(1, 3 * C).astype(BF16)

    in_maps = []
    for b in range(B):
        mask_b = attention_mask[b].astype(np.float32).astype(BF16)
        for g in range(G):
            in_maps.append(
                {
                    "xqT": np.ascontiguousarray(inputs_q[b, g].T).astype(BF16),
                    "xkvT": np.ascontiguousarray(inputs_kv[b, g].T).astype(BF16),
                    "maskq": mask_b,
                    "W4": w4,
                    "B3": b3,
                }
            )
    return in_maps


def kernel(
    inputs_q, inputs_kv, attention_mask, Wq, bq, Wk, bk, Wv, bv, Wo, bo
):
    from concourse import bass_utils

    inputs_q = np.asarray(inputs_q, dtype=np.float32)
    inputs_kv = np.asarray(inputs_kv, dtype=np.float32)
    attention_mask = np.asarray(attention_mask)
    Wq, bq = np.asarray(Wq, np.float32), np.asarray(bq, np.float32)
    Wk, bk = np.asarray(Wk, np.float32), np.asarray(bk, np.float32)
    Wv, bv = np.asarray(Wv, np.float32), np.asarray(bv, np.float32)
    Wo, bo = np.asarray(Wo, np.float32), np.asarray(bo, np.float32)

    in_maps = _make_in_maps(
        inputs_q, inputs_kv, attention_mask, Wq, bq, Wk, bk, Wv, bv, Wo
    )
    nc = _get_nc()
    res = bass_utils.run_bass_kernel_spmd(nc, in_maps, core_ids=list(range(8)))

    final = np.empty((B, G, QL, C), np.float32)
    probs = np.empty((B, G, H, QL, KL), np.float32)
    for b in range(B):
        for g in range(G):
            r = res.results[b * G + g]
            final[b, g] = r["out_final"] + bo[None, :]
            probs[b, g] = r["out_probs"]
    return final, probs


# revision 35
# speedup vs baseline: 1.9186x; 1.9186x over previous
"""Trainium2 Bass kernel for nn_Attention_24919400252009.

Multi-head attention (8 heads, head_dim 32) over B=2,G=4,Q=K=1024,C=256,
returning (final_output, attn_probs).  Data-parallel across the 8
NeuronCores: one (b, g) pair per core, no collectives.

Per-core pipeline (bf16 compute, f32 accumulation), software-pipelined
across heads (stage T/D/A of head h-1 is emitted with stage S/E/M/N of
head h so every engine's in-order stream stays dense):
  S: PE scores_qk = qT_h.T @ kT_h    (contraction d=32, row-group packed)
  E: ACT e = exp(scores / sqrt(32))  (PSUM -> SBUF bf16)
  M: DVE scalar_tensor_tensor: em = e * mask, row-sums accum (f32)
  N: DVE tensor_scalar: probs = em * (1/sum)  (bf16, in place)
  T: HWDGE xbar transpose probs [q,k] -> [k,q] bf16
  D: SWDGE cast-DMA probs bf16 -> HBM f32 (attn_probs output)
  A: PE out_h = v_h.T @ probsT (col-group packed) -> Wo projection
"""

import sys

if "/opt/trn_rl_repo" not in sys.path:
    sys.path.insert(0, "/opt/trn_rl_repo")

import numpy as np
import ml_dtypes

B, G, QL, KL = 2, 4, 1024, 1024
C = 256          # CQ = CKV = QK_CH = V_CH = OUT_CH
H = 8            # num heads
HD = 32          # head dim
P = 128          # partitions
NQT = QL // P    # 8 q tiles
NKT = KL // P    # 8 k tiles
NH = QL // 512   # matmul free-dim halves (PSUM bank limit N <= 512)
SCALE = 1.0 / np.sqrt(HD)

BF16 = ml_dtypes.bfloat16

_CACHED = {}


def _build_nc(repeats=1, serial=False, accum_bf16=False, no_accum=False,
              tp_batch=1):
    import concourse.mybir as mybir
    import concourse.tile as tile
    from concourse import bacc
    from concourse.bass import ts

    nc = bacc.Bacc()
    bf = mybir.dt.bfloat16
    f32 = mybir.dt.float32

    # ---- DRAM I/O (per-core shard shapes) ----
    xqT_d = nc.dram_tensor("xqT", [C, QL], bf, kind="ExternalInput")
    xkvT_d = nc.dram_tensor("xkvT", [C, KL], bf, kind="ExternalInput")
    mask_d = nc.dram_tensor("maskq", [QL, KL], bf, kind="ExternalInput")
    w4_d = nc.dram_tensor("W4", [C, 4 * C], bf, kind="ExternalInput")  # q,k,v,o
    b3_d = nc.dram_tensor("B3", [1, 3 * C], bf, kind="ExternalInput")  # q,k,v
    outf_d = nc.dram_tensor("out_final", [QL, C], f32, kind="ExternalOutput")
    # probs leave the device as bf16 (halves HBM write traffic); the host
    # widens to f32 — identical values, since probs are computed in bf16
    outp_d = nc.dram_tensor("out_probs", [H, QL, KL], bf, kind="ExternalOutput")

    with tile.TileContext(nc) as tc:
        with (
            tc.tile_pool(name="const", bufs=1) as cpool,
            tc.tile_pool(name="work", bufs=2) as wpool,
            tc.tile_pool(name="psum", bufs=2, space="PSUM") as ppool,
        ):
            # ---- load inputs (HWDGE; mask last - not needed until M(0)) ----
            xqT = cpool.tile([P, 2, QL], bf)       # [c_in, c_chunk, q]
            xkvT = cpool.tile([P, 2, KL], bf)
            w4 = cpool.tile([P, 2, 4 * C], bf)     # [c_in, c_chunk, 4*ch_out]
            b3 = cpool.tile([1, 3 * C], bf)
            mask = cpool.tile([P, NQT, KL], bf)    # [q_in_tile, q_tile, k]
            ones = cpool.tile([1, QL], bf)

            nc.sync.dma_start(xqT[:], xqT_d[:].rearrange("(cc p) q -> p cc q", p=P))
            nc.sync.dma_start(w4[:], w4_d[:].rearrange("(cc p) o -> p cc o", p=P))
            nc.sync.dma_start(xkvT[:], xkvT_d[:].rearrange("(cc p) q -> p cc q", p=P))
            nc.sync.dma_start(b3[:], b3_d[:])
            nc.sync.dma_start(mask[:], mask_d[:].rearrange("(qt p) k -> p qt k", p=P))
            nc.vector.memset(ones[:], 1.0)

            wqT = w4[:, :, 0 * C : 1 * C]
            wkT = w4[:, :, 1 * C : 2 * C]
            wvT = w4[:, :, 2 * C : 3 * C]
            woT = w4[:, :, 3 * C : 4 * C]

            # ---- projections ----
            # qT/kT: [ch_out(2x128), q] = W @ x.T (+ bias via rank-1 matmul)
            qT = cpool.tile([P, 2, QL], bf)
            kT = cpool.tile([P, 2, KL], bf)
            for i, (dst, w, x) in enumerate(((qT, wqT, xqT), (kT, wkT, xkvT))):
                for oc in range(2):  # output-channel chunk
                    ps_p = ppool.tile([P, QL], mybir.dt.float32, tag="ps_a")
                    for nh in range(NH):
                        for cc in range(2):  # contraction chunk
                            nc.tensor.matmul(
                                ps_p[:, ts(nh, 512)],
                                w[:, cc, ts(oc, P)],
                                x[:, cc, ts(nh, 512)],
                                start=(cc == 0),
                                stop=False,
                            )
                        nc.tensor.matmul(
                            ps_p[:, ts(nh, 512)],
                            b3[:, i * C + oc * P : i * C + (oc + 1) * P],
                            ones[:, ts(nh, 512)],
                            start=False, stop=True,
                        )
                    nc.scalar.copy(dst[:, oc, :], ps_p[:])

            # v: [k(8x128), ch_out] = x_kv @ Wv.T (+ bias via rank-1 matmul)
            v = cpool.tile([P, NKT, C], bf)
            for kt in range(NKT):
                ps_v = ppool.tile([P, QL], mybir.dt.float32, tag="ps_a")
                for cc in range(2):
                    nc.tensor.matmul(
                        ps_v[:, :C],
                        xkvT[:, cc, ts(kt, P)],
                        wvT[:, cc, :],
                        start=(cc == 0),
                        stop=False,
                    )
                nc.tensor.matmul(
                    ps_v[:, :C], ones[:, :P], b3[:, 2 * C : 2 * C + C],
                    start=False, stop=True,
                )
                nc.scalar.copy(v[:, kt, :], ps_v[:, :C])

            # ---- attention: software-pipelined head loop ----
            outT = cpool.tile([P, 2, QL], bf)  # [d(4x32 per chunk), chunk, q]
            state = {}

            def stage_semn(h):
                """scores -> exp -> mask+sums -> recip -> normalize.

                recip/normalize run per half-head so the downstream
                transpose + output DMA can start before the second half's
                mask pass finishes.
                """
                hc, hp = h // 4, (h % 4) * HD
                hq = NQT // 2
                e = wpool.tile([P, NQT, KL], bf, tag="e", name=f"e{h}", bufs=3)
                sum_dt = bf if accum_bf16 else mybir.dt.float32
                sums = wpool.tile([P, NQT], sum_dt, tag="sums",
                                  name=f"sums{h}")
                rinv = wpool.tile([P, NQT], mybir.dt.float32, tag="rinv",
                                  name=f"rinv{h}")
                for qt in range(NQT):
                    ps_s = ppool.tile([P, KL], mybir.dt.float32, tag="ps_a",
                                      name=f"ps_s{h}_{qt}")
                    for nh in range(NH):
                        nc.tensor.matmul(
                            ps_s[:, ts(nh, 512)],
                            qT[hp : hp + HD, hc, ts(qt, P)],
                            kT[hp : hp + HD, hc, ts(nh, 512)],
                            start=True, stop=True,
                            tile_position=(hp, 0),
                        )
                    nc.scalar.activation(
                        e[:, qt, :], ps_s[:],
                        mybir.ActivationFunctionType.Exp,
                        scale=float(SCALE),
                    )
                    if no_accum:
                        # timing variant: plain tensor_tensor (2x bf16 mode),
                        # sums filled with 1.0 (wrong values, same structure)
                        nc.vector.tensor_tensor(
                            e[:, qt, :], e[:, qt, :], mask[:, qt, :],
                            mybir.AluOpType.mult,
                        )
                        nc.vector.memset(sums[:, qt : qt + 1], 1.0)
                    else:
                        nc.vector.scalar_tensor_tensor(
                            out=e[:, qt, :],
                            in0=e[:, qt, :],
                            scalar=1.0,
                            in1=mask[:, qt, :],
                            op0=mybir.AluOpType.mult,
                            op1=mybir.AluOpType.mult,
                            accum_out=sums[:, qt : qt + 1],
                        )
                    if qt % hq == hq - 1:
                        ch = qt // hq
                        sl = slice(ch * hq, (ch + 1) * hq)
                        nc.vector.reciprocal(rinv[:, sl], sums[:, sl])
                        for q2 in range(ch * hq, (ch + 1) * hq):
                            nc.vector.tensor_scalar_mul(
                                e[:, q2, :], e[:, q2, :],
                                rinv[:, q2 : q2 + 1],
                            )
                return e

            def stage_td(h, e):
                """xbar transposes + attn_probs cast-DMA out."""
                pT = wpool.tile([P, NQT, NKT, P], bf, tag="pT", name=f"pT{h}",
                                bufs=3)
                hq = NQT // 2
                for qt in range(NQT):
                    if qt % tp_batch == tp_batch - 1:
                        q0 = qt - tp_batch + 1
                        nc.sync.dma_start_transpose(
                            pT[:, q0 : qt + 1, :, :], e[:, q0 : qt + 1, :]
                        )
                    if qt % hq == hq - 1:
                        # attn_probs out (bf16, SWDGE)
                        ch = qt // hq
                        nc.gpsimd.dma_start(
                            outp_d[h][ch * hq * P : (ch + 1) * hq * P].rearrange(
                                "(qt p) k -> p qt k", p=P
                            ),
                            e[:, ch * hq : (ch + 1) * hq, :],
                        )
                return pT

            def stage_a(h, pT):
                """AV matmuls (per q-block accumulation) + group drain."""
                hc, hp = h // 4, (h % 4) * HD
                if h % 4 == 0:
                    state["ps_o"] = ppool.tile(
                        [P, QL], mybir.dt.float32, tag="ps_o", name=f"ps_o{hc}"
                    )
                ps_o = state["ps_o"]
                for qt in range(NQT):
                    for kt in range(NKT):
                        nc.tensor.matmul(
                            ps_o[hp : hp + HD, ts(qt, P)],
                            v[:, kt, h * HD : (h + 1) * HD],
                            pT[:, qt, kt, :],
                            start=(kt == 0), stop=(kt == NKT - 1),
                            tile_position=(0, hp),
                        )
                if h % 4 == 3:
                    nc.scalar.copy(outT[:, hc, :], ps_o[:])

            for _rep in range(repeats):
                if serial and _rep:
                    tc.strict_bb_all_engine_barrier()
                for i in range(H + 2):
                    if i < H:
                        state[("e", i)] = stage_semn(i)
                    if 1 <= i <= H:
                        state[("pT", i - 1)] = stage_td(
                            i - 1, state.pop(("e", i - 1))
                        )
                    if i >= 2:
                        stage_a(i - 2, state.pop(("pT", i - 2)))

                # ---- final projection: outF[q, ch] = outT.T @ WoT ----
                outF = cpool.tile([P, NQT, C], f32, tag="outF")
                for qt in range(NQT):
                    ps_f = ppool.tile([P, QL], mybir.dt.float32, tag="ps_a")
                    for cc in range(2):
                        nc.tensor.matmul(
                            ps_f[:, :C],
                            outT[:, cc, ts(qt, P)],
                            woT[:, cc, :],
                            start=(cc == 0),
                            stop=(cc == 1),
                        )
                    nc.scalar.copy(outF[:, qt, :], ps_f[:, :C])
                nc.gpsimd.dma_start(
                    outf_d[:].rearrange("(qt p) c -> p qt c", p=P), outF[:]
                )

    nc.compile()
    return nc


def _get_nc():
    if "nc" not in _CACHED:
        _CACHED["nc"] = _build_nc()
    return _CACHED["nc"]


def _make_in_maps(inputs_q, inputs_kv, attention_mask, Wq, bq, Wk, bk, Wv, bv, Wo):
    w4 = np.concatenate([Wq.T, Wk.T, Wv.T, Wo.T], axis=1)
    w4 = np.ascontiguousarray(w4).astype(BF16)  # [c_in, 4*ch_out]
    b3 = np.concatenate([bq, bk, bv]).reshape(1, 3 * C).astype(BF16)

    in_maps = []
    for b in range(B):
        mask_b = attention_mask[b].astype(np.float32).astype(BF16)
        for g in range(G):
            in_maps.append(
                {
                    "xqT": np.ascontiguousarray(inputs_q[b, g].T).astype(BF16),
                    "xkvT": np.ascontiguousarray(inputs_kv[b, g].T).astype(BF16),
                    "maskq": mask_b,
                    "W4": w4,
                    "B3": b3,
                }
            )
    return in_maps


def kernel(
    inputs_q, inputs_kv, attention_mask, Wq, bq, Wk, bk, Wv, bv, Wo, bo
):
    from concourse import bass_utils

    inputs_q = np.asarray(inputs_q, dtype=np.float32)
    inputs_kv = np.asarray(inputs_kv, dtype=np.float32)
    attention_mask = np.asarray(attention_mask)
    Wq, bq = np.asarray(Wq, np.float32), np.asarray(bq, np.float32)
    Wk, bk = np.asarray(Wk, np.float32), np.asarray(bk, np.float32)
    Wv, bv = np.asarray(Wv, np.float32), np.asarray(bv, np.float32)
    Wo, bo = np.asarray(Wo, np.float32), np.asarray(bo, np.float32)

    in_maps = _make_in_maps(
        inputs_q, inputs_kv, attention_mask, Wq, bq, Wk, bk, Wv, bv, Wo
    )
    nc = _get_nc()
    res = bass_utils.run_bass_kernel_spmd(nc, in_maps, core_ids=list(range(8)))

    final = np.empty((B, G, QL, C), np.float32)
    probs = np.empty((B, G, H, QL, KL), np.float32)
    for b in range(B):
        for g in range(G):
            r = res.results[b * G + g]
            final[b, g] = r["out_final"] + bo[None, :]
            probs[b, g] = r["out_probs"].astype(np.float32)
    return final, probs
